# revision 1
# baseline (speedup 1.0000x reference)
"""Multi-head attention (B=2, T=2048, D=1024, H=16, causal) on 8 TRN2 NeuronCores.

Sharding (tensor-parallel heads + token-parallel epilogue):
  - Core c owns heads (2c, 2c+1) -> a 128-wide slice of the QKV output dim.
  - QKV projections: qT/kT/vT [128, B*T] feature-major, from a host-re-tiled
    x^T (one contiguous 16KB descriptor per partition per token slab) and
    host-pre-transposed weight slices (float32r matmuls, 1 cyc/row).
  - Attention: streaming over 128-wide key blocks, transposed score tiles
    S^T [k, q] for both heads in one [128, 1024] PSUM pair; causal mask is a
    -300 staircase *bias* accumulated by an identity-stationary matmul (exp
    of masked entries underflows to 0); one wide exp per k-block on ScalarE;
    ctx^T accumulates with an appended ones-column in v so row 64 of the
    accumulator is the softmax denominator.  The k-loop is software-pipelined
    (ctx of block k issues after scores of block k+1).
  - Emission interleaves batch-1 projections into batch-0 attention so the
    TensorE stream stays dense and ScalarE is never the only busy engine.
  - AllToAll over token slices redistributes ctx^T (2 MB/core minimal
    exchange); output projection is token-sharded; host concatenates.
"""

import numpy as np

import concourse.bacc as bacc
import concourse.bass as bass
import concourse.mybir as mybir
import concourse.tile as tile
from concourse import bass_utils
from concourse.bass import ts

D = 1024
H = 16
DK = D // H  # 64
NCORES = 8
HPC = H // NCORES  # heads per core = 2
DSL = HPC * DK  # per-core QKV output slice = 128
P = 128
QBLK = 512
KBLK = 128
DA = DK + 1  # 65: head dim + ones column (softmax denominator row)

F32 = mybir.dt.float32
F32R = mybir.dt.float32r
EXP = mybir.ActivationFunctionType.Exp
IDENT = mybir.ActivationFunctionType.Identity


def build_nc(B=2, T=2048):
    """Build the SPMD Bass module (identical program on all 8 cores)."""
    NTOK = B * T
    TPC = NTOK // NCORES  # tokens per core in the output projection
    KO = D // P  # 8 contraction chunks
    NKB = T // KBLK  # key blocks per batch
    NQB = T // QBLK  # query blocks per batch
    TB = TPC // P  # 128-token sub-blocks in the output projection
    NSLAB = NTOK // QBLK  # x token slabs
    NPAIR = NSLAB // 2

    nc = bacc.Bacc("TRN2", target_bir_lowering=False, debug=False,
                   num_devices=NCORES)

    # ---- DRAM I/O ------------------------------------------------------
    xT_d = nc.dram_tensor("xT", [P, NSLAB, KO, QBLK], F32R, kind="ExternalInput")
    wqT_d = nc.dram_tensor("wqT", [D, DSL], F32R, kind="ExternalInput")
    wkT_d = nc.dram_tensor("wkT", [D, DSL], F32R, kind="ExternalInput")
    wvT_d = nc.dram_tensor("wvT", [D, DSL], F32R, kind="ExternalInput")
    woT_d = nc.dram_tensor("woT", [D, D], F32R, kind="ExternalInput")
    bq_d = nc.dram_tensor("bq", [DSL, 1], F32, kind="ExternalInput")
    bk_d = nc.dram_tensor("bk", [DSL, 1], F32, kind="ExternalInput")
    bv_d = nc.dram_tensor("bv", [DSL, 1], F32, kind="ExternalInput")
    bo_d = nc.dram_tensor("bo", [D], F32, kind="ExternalInput")
    mask_d = nc.dram_tensor("mask", [P, 2 * QBLK - KBLK], F32R,
                            kind="ExternalInput")
    ident_d = nc.dram_tensor("ident", [P, P], F32R, kind="ExternalInput")
    ones_d = nc.dram_tensor("ones", [P, P], F32R, kind="ExternalInput")
    out_d = nc.dram_tensor("out", [TPC, D], F32, kind="ExternalOutput")

    with tile.TileContext(nc) as tc:
        with (
            tc.tile_pool(name="consts", bufs=1) as consts,
            tc.tile_pool(name="acts", bufs=1) as acts,
            tc.tile_pool(name="xin", bufs=3) as xin,
            tc.tile_pool(name="attn", bufs=2) as attn_pool,
            tc.tile_pool(name="small", bufs=1) as small,
            tc.tile_pool(name="outg", bufs=1) as outg,
            tc.tile_pool(name="outp", bufs=1) as outp,
            tc.tile_pool(name="psA", bufs=2, space="PSUM") as psA,
            tc.tile_pool(name="psC", bufs=2, space="PSUM") as psC,
            tc.tile_pool(name="dram", bufs=2, space="DRAM") as dram,
        ):
            # ---- small constants -----------------------------------
            bq_sb = consts.tile([P, 1], F32, tag="bq")
            bk_sb = consts.tile([P, 1], F32, tag="bk")
            bv_sb = consts.tile([P, 1], F32, tag="bv")
            nc.sync.dma_start(bq_sb[:], bq_d.ap())
            nc.sync.dma_start(bk_sb[:], bk_d.ap())
            nc.sync.dma_start(bv_sb[:], bv_d.ap())
            mask_sb = consts.tile([P, 2 * QBLK - KBLK], F32R, tag="mask")
            nc.sync.dma_start(mask_sb[:], mask_d.ap())
            ident_sb = consts.tile([P, P], F32R, tag="ident")
            nc.sync.dma_start(ident_sb[:], ident_d.ap())

            # QKV weights, loaded per-ko chunk so the first matmuls can
            # start after ~64KB instead of the full 1.5MB.
            wq_sb = consts.tile([P, KO, DSL], F32R, tag="wq")
            wk_sb = consts.tile([P, KO, DSL], F32R, tag="wk")
            wv_sb = consts.tile([P, KO, DSL], F32R, tag="wv")
            for ko in range(KO):
                for w_sb, w_d in ((wq_sb, wqT_d), (wk_sb, wkT_d),
                                  (wv_sb, wvT_d)):
                    nc.sync.dma_start(
                        w_sb[:, ko],
                        w_d.ap().rearrange("(ko p) m -> p ko m", p=P)[:, ko])

            qT = acts.tile([P, NTOK], F32R, tag="qT")
            kT = acts.tile([P, NTOK], F32R, tag="kT")
            vT = acts.tile([P, NTOK], F32R, tag="vT")
            v_nat = acts.tile([P, NTOK // P, 2 * DA], F32R, tag="v_nat")
            nc.sync.dma_start(v_nat[:, :, DK], ones_d.ap()[:, 0:NTOK // P])
            nc.sync.dma_start(v_nat[:, :, DA + DK], ones_d.ap()[:, 0:NTOK // P])

            def proj_pair(i):
                """QKV projections for token slabs 2i, 2i+1 (one stationary
                load per (proj, ko), wide PSUM + one wide epilogue ACT)."""
                xt0 = xin.tile([P, KO, QBLK], F32R, tag="xt", name="xt0")
                xt1 = xin.tile([P, KO, QBLK], F32R, tag="xt", name="xt1")
                nc.sync.dma_start(xt0[:], xT_d.ap()[:, 2 * i])
                nc.sync.dma_start(xt1[:], xT_d.ap()[:, 2 * i + 1])
                for w_sb, b_sb, dst in ((wq_sb, bq_sb, qT),
                                        (wk_sb, bk_sb, kT),
                                        (wv_sb, bv_sb, vT)):
                    ps = psA.tile([P, 2 * QBLK], F32, tag="sp", name="ps")
                    for ko in range(KO):
                        nc.tensor.matmul(ps[:, 0:QBLK], w_sb[:, ko],
                                         xt0[:, ko], start=(ko == 0),
                                         stop=(ko == KO - 1))
                        nc.tensor.matmul(ps[:, QBLK:], w_sb[:, ko],
                                         xt1[:, ko], start=(ko == 0),
                                         stop=(ko == KO - 1))
                    nc.scalar.activation(dst[:, ts(i, 2 * QBLK)], ps[:],
                                         IDENT, bias=b_sb[:, 0:1])

            def v_nat_block(j):
                """Transpose one [128,128] vT tile into v_nat (both heads),
                leaving the ones columns intact."""
                ptf = psA.tile([P, 2 * QBLK], F32R, tag="sp", name="ptf")
                pt = ptf[:, :P]
                nc.tensor.transpose(pt[:], vT[:, ts(j, P)], ident_sb[:])
                nc.vector.tensor_copy(v_nat[:, j, 0:DK], pt[:, 0:DK])
                nc.vector.tensor_copy(v_nat[:, j, DA:DA + DK], pt[:, DK:P])

            a2a_in = dram.tile([NCORES, P, TPC], F32R, tag="a2a_in")
            a2a_out = dram.tile([NCORES, P, TPC], F32R, tag="a2a_out")

            def attention_qblock(b, qi):
                q_sl = ts(b * T // QBLK + qi, QBLK)
                nkb = (qi + 1) * (QBLK // KBLK)
                C0 = psC.tile([P, QBLK], F32, tag="ctx0", name="C0")
                C1 = psC.tile([P, QBLK], F32, tag="ctx1", name="C1")

                def emit_ctx(pend):
                    ap_, jjp, st, sp = pend
                    nc.tensor.matmul(C0[0:DA], v_nat[:, jjp, 0:DA],
                                     ap_[:, 0:QBLK], start=st, stop=sp)
                    nc.tensor.matmul(C1[0:DA], v_nat[:, jjp, DA:2 * DA],
                                     ap_[:, QBLK:], start=st, stop=sp)

                pend = None
                for ki in range(nkb):
                    k_sl = ts(b * T // KBLK + ki, KBLK)
                    jj = b * NKB + ki
                    doff = ki * KBLK - qi * QBLK
                    diag = doff >= 0
                    sp_t = psA.tile([P, 2 * QBLK], F32, tag="sp", name="sp_t")
                    nc.tensor.matmul(sp_t[:, 0:QBLK],
                                     kT[0:DK, k_sl], qT[0:DK, q_sl],
                                     start=True, stop=not diag,
                                     tile_position=(0, 0))
                    nc.tensor.matmul(sp_t[:, QBLK:],
                                     kT[DK:P, k_sl], qT[DK:P, q_sl],
                                     start=True, stop=not diag,
                                     tile_position=(64, 0))
                    if diag:
                        # causal staircase bias (-300 where masked)
                        s = QBLK - KBLK - doff
                        m = mask_sb[:, s:s + QBLK]
                        nc.tensor.matmul(sp_t[:, 0:QBLK], ident_sb[:], m,
                                         start=False, stop=True)
                        nc.tensor.matmul(sp_t[:, QBLK:], ident_sb[:], m,
                                         start=False, stop=True)
                    a_p = attn_pool.tile([P, 2 * QBLK], F32R, tag="ap",
                                         name="a_p")
                    nc.scalar.activation(a_p[:], sp_t[:], EXP)
                    # software pipeline: ctx of the previous k-block issues
                    # after this block's scores, so PE runs ahead of ACT.
                    if pend is not None:
                        emit_ctx(pend)
                    pend = (a_p, jj, ki == 0, ki == nkb - 1)
                emit_ctx(pend)

                # normalize ctx^T by 1/denominator (row 64), partition-
                # broadcast the reciprocal via a DRAM bounce.
                rec = small.tile([P, 2 * QBLK], F32, tag="rec")
                nc.vector.reciprocal(rec[DK:DA, 0:QBLK], C0[DK:DA])
                nc.vector.reciprocal(rec[DK:DA, QBLK:], C1[DK:DA])
                rec_dr = dram.tile([1, 2 * QBLK], F32, tag="rec_dr",
                                   name="rec_dr")
                nc.sync.dma_start(rec_dr[:], rec[DK:DA, :])
                rb_sb = small.tile([P, 2 * QBLK], F32, tag="rb_sb")
                nc.sync.dma_start(rb_sb[0:DK, :],
                                  rec_dr[:].to_broadcast((DK, 2 * QBLK)))
                ctx0_sb = small.tile([P, QBLK], F32R, tag="ctx0_sb")
                ctx1_sb = small.tile([P, QBLK], F32R, tag="ctx1_sb")
                nc.vector.tensor_mul(ctx0_sb[0:DK], C0[0:DK],
                                     rb_sb[0:DK, 0:QBLK])
                nc.vector.tensor_mul(ctx1_sb[0:DK], C1[0:DK],
                                     rb_sb[0:DK, QBLK:])
                assert QBLK % TPC == 0
                for sub in range(QBLK // TPC):
                    chunk = (b * T + qi * QBLK) // TPC + sub
                    nc.sync.dma_start(a2a_in[chunk, 0:DK],
                                      ctx0_sb[0:DK, ts(sub, TPC)])
                    nc.sync.dma_start(a2a_in[chunk, DK:P],
                                      ctx1_sb[0:DK, ts(sub, TPC)])

            # ---- phase plan: batch-0 proj -> batch-0 attention while
            # batch-1 proj/v_nat fill PE gaps -> batch-1 attention --------
            half_pairs = NPAIR // B  # proj pairs per batch
            for i in range(half_pairs):
                proj_pair(i)
            for j in range(NTOK // P // B):
                v_nat_block(j)

            # wide constants for the tail, loaded mid-kernel so they don't
            # fight the startup DMA burst
            wo_sb = consts.tile([P, KO, D], F32R, tag="wo")
            bo_sb = consts.tile([P, D], F32, tag="bo")

            late = []
            for i in range(half_pairs, NPAIR):
                late.append(lambda i=i: proj_pair(i))
            late.append(lambda: nc.sync.dma_start(
                wo_sb[:], woT_d.ap().rearrange("(ko p) m -> p ko m", p=P)))
            late.append(lambda: nc.sync.dma_start(
                bo_sb[:], bo_d.ap()[None, :].to_broadcast((P, D))))
            for j0 in range(NTOK // P // B, NTOK // P, 4):
                late.append(lambda j0=j0: [v_nat_block(j)
                                           for j in range(j0, j0 + 4)])

            for qi in range(NQB):
                attention_qblock(0, qi)
                # interleave deferred batch-1 work into batch-0 attention
                nlate = max(1, (len(late) + NQB - 1 - qi) // (NQB - qi))
                for _ in range(min(nlate, len(late))):
                    late.pop(0)()
            while late:
                late.pop(0)()
            for qi in range(NQB):
                attention_qblock(1, qi)

            # ---- AllToAll over token slices -----------------------------
            nc.gpsimd.collective_compute(
                "AllToAll",
                mybir.AluOpType.bypass,
                replica_groups=[list(range(NCORES))],
                ins=[a2a_in[:].opt()],
                outs=[a2a_out[:].opt()],
            )

            # ---- output projection (token-sharded) ----------------------
            ctxg = outg.tile([P, KO, TPC], F32R, tag="ctxg")
            nc.sync.dma_start(ctxg[:], a2a_out[:].rearrange("j p t -> p j t"))
            for tb in range(TB):
                po = psA.tile([P, 2 * QBLK], F32, tag="sp", name="po")
                for ko in range(KO):
                    nc.tensor.matmul(po[:, 0:QBLK], ctxg[:, ko, ts(tb, P)],
                                     wo_sb[:, ko, 0:QBLK],
                                     start=(ko == 0), stop=(ko == KO - 1))
                    nc.tensor.matmul(po[:, QBLK:], ctxg[:, ko, ts(tb, P)],
                                     wo_sb[:, ko, QBLK:],
                                     start=(ko == 0), stop=(ko == KO - 1))
                o_sb = outp.tile([P, D], F32, tag="o_sb")
                nc.vector.tensor_add(o_sb[:], po[:], bo_sb[:])
                nc.sync.dma_start(out_d.ap()[ts(tb, P), :], o_sb[:])

    nc.compile()
    return nc


_NC_CACHE = {}


def _get_nc(B, T):
    key = (B, T)
    if key not in _NC_CACHE:
        _NC_CACHE[key] = build_nc(B, T)
    return _NC_CACHE[key]


def make_in_maps(x, Wq, bq, Wk, bk, Wv, bv, Wo, bo):
    B, T, _ = x.shape
    NTOK = B * T
    NSLAB = NTOK // QBLK
    KO = D // P
    x = np.asarray(x, np.float32)
    # [D, NTOK] -> [p, slab, ko, t]: one contiguous 16KB DMA descriptor per
    # partition per slab.
    xT = x.reshape(NTOK, D).T  # [D, NTOK]
    xT_t = np.ascontiguousarray(
        xT.reshape(KO, P, NSLAB, QBLK).transpose(1, 2, 0, 3))
    woT = np.ascontiguousarray(np.asarray(Wo, np.float32).T)
    bo = np.asarray(bo, np.float32)
    # causal staircase bias: 0 where allowed (c >= kk + (QBLK-KBLK)),
    # -300 where masked; accumulated into scores via an identity-stationary
    # matmul so exp() of masked entries underflows to zero.
    keep = (np.arange(2 * QBLK - KBLK)[None, :]
            >= (np.arange(P)[:, None] + (QBLK - KBLK)))
    mask = np.where(keep, 0.0, -300.0).astype(np.float32)
    ident = np.eye(P, dtype=np.float32)
    ones = np.ones((P, P), np.float32)
    in_maps = []
    for c in range(NCORES):
        sl = slice(DSL * c, DSL * (c + 1))
        in_maps.append({
            "xT": xT_t,
            "wqT": np.ascontiguousarray(np.asarray(Wq, np.float32)[sl].T) * 0.125,
            "wkT": np.ascontiguousarray(np.asarray(Wk, np.float32)[sl].T),
            "wvT": np.ascontiguousarray(np.asarray(Wv, np.float32)[sl].T),
            "woT": woT,
            "bq": (np.asarray(bq, np.float32)[sl] * 0.125).reshape(DSL, 1),
            "bk": np.asarray(bk, np.float32)[sl].reshape(DSL, 1),
            "bv": np.asarray(bv, np.float32)[sl].reshape(DSL, 1),
            "bo": bo,
            "mask": mask,
            "ident": ident,
            "ones": ones,
        })
    return in_maps


LAST_RESULTS = None


def kernel(x, Wq, bq, Wk, bk, Wv, bv, Wo, bo, trace=False, trace_cores=None):
    global LAST_RESULTS
    B, T, _ = x.shape
    nc = _get_nc(B, T)
    in_maps = make_in_maps(x, Wq, bq, Wk, bk, Wv, bv, Wo, bo)
    kw = {}
    if trace:
        kw = dict(trace=True, trace_cores=trace_cores)
    res = bass_utils.run_bass_kernel_spmd(nc, in_maps,
                                          core_ids=list(range(NCORES)), **kw)
    LAST_RESULTS = res
    out = np.concatenate([res.results[c]["out"] for c in range(NCORES)], axis=0)
    return out.reshape(B, T, D)



# revision 8
# speedup vs baseline: 1.4456x; 1.4456x over previous
"""Multi-head attention (B=2, T=2048, D=1024, H=16, causal) on 8 TRN2 NeuronCores.

Sharding (tensor-parallel heads + token-parallel epilogue):
  - Core c owns heads (2c, 2c+1) -> a 128-wide slice of the QKV output dim.
  - bf16 operands everywhere on the matmul path (f32 PSUM accumulation);
    x / weights are cast to bf16 on the host, halving HBM traffic.
  - Startup DMAs are issue-rate bound (~650ns per dma_start on a queue), so
    the critical path uses few, large dma_starts spread over three issue
    queues (sync: x slabs, scalar: weights, gpsimd: small constants).
  - QKV projections: qT/kT [128, B*T] bf16 feature-major; bias epilogues on
    VectorE (tensor_scalar_add) so ScalarE is reserved for attention exp.
  - Attention: streaming over 128-wide key blocks, transposed score tiles
    S^T [k, q] for both heads in one [128, 1024] PSUM pair; on diagonal
    blocks only the valid suffix [doff, 512) is computed and a 128-wide
    triangular -300 bias tile is accumulated on top (exp underflows to 0);
    one (narrowed) exp per k-block on ScalarE; ctx^T accumulates with an
    appended ones-column in v so row 64 of the accumulator is the softmax
    denominator.  The k-loop is software-pipelined (ctx of block k issues
    after scores of block k+1).
  - Softmax normalize: denominator rows leave PSUM via ScalarE copies, are
    DMA-broadcast across partitions through a DRAM bounce, inverted with
    reciprocal_approx_fast, and applied as wide [64, N] VectorE multiplies.
  - The ctx AllToAll is split into FOUR collectives (2 batches x 2 token
    groups), each triggered as soon as its qblocks finish, so exchanges and
    the token-sharded output projection overlap attention compute; only the
    last quarter remains in the tail.  Host reassembles the token layout.
"""

import numpy as np
import ml_dtypes

import concourse.bacc as bacc
import concourse.bass as bass
import concourse.mybir as mybir
import concourse.tile as tile
from concourse import bass_utils
from concourse.bass import ts

D = 1024
H = 16
DK = D // H  # 64
NCORES = 8
HPC = H // NCORES  # heads per core = 2
DSL = HPC * DK  # per-core QKV output slice = 128
P = 128
QBLK = 512
KBLK = 128
DA = DK + 1  # 65: head dim + ones column (softmax denominator row)
G = 2  # token groups per batch (collective split granularity)

F32 = mybir.dt.float32
F32R = mybir.dt.float32r
BF16 = mybir.dt.bfloat16
EXP = mybir.ActivationFunctionType.Exp

BF16_NP = ml_dtypes.bfloat16


def build_nc(B=2, T=2048):
    """Build the SPMD Bass module (identical program on all 8 cores)."""
    NTOK = B * T
    TPC = NTOK // NCORES  # tokens per core in the output projection
    TPB = T // NCORES  # tokens per core per batch
    TPG = TPB // G  # tokens per core per (batch, group)
    KO = D // P  # 8 contraction chunks
    NKB = T // KBLK  # key blocks per batch
    NQB = T // QBLK  # query blocks per batch
    GQ = NQB // G  # q-blocks per group
    NSLAB = NTOK // QBLK  # x token slabs
    NPAIR = NSLAB // 2
    NDST = QBLK // TPG  # destination cores covered by one q-block
    TW = min(P, TPG)  # outproj token-tile width
    TBG = TPG // TW  # outproj token tiles per (batch, group)

    nc = bacc.Bacc("TRN2", target_bir_lowering=False, debug=False,
                   num_devices=NCORES)

    # ---- DRAM I/O ------------------------------------------------------
    xT_d = nc.dram_tensor("xT", [P, NPAIR, KO, 2 * QBLK], BF16,
                          kind="ExternalInput")
    wqT_d = nc.dram_tensor("wqT", [D, DSL], BF16, kind="ExternalInput")
    wkT_d = nc.dram_tensor("wkT", [D, DSL], BF16, kind="ExternalInput")
    wvT_d = nc.dram_tensor("wvT", [D, DSL], BF16, kind="ExternalInput")
    woT_d = nc.dram_tensor("woT", [D, D], BF16, kind="ExternalInput")
    bq_d = nc.dram_tensor("bq", [DSL, 1], F32, kind="ExternalInput")
    bk_d = nc.dram_tensor("bk", [DSL, 1], F32, kind="ExternalInput")
    bv_d = nc.dram_tensor("bv", [DSL, 1], F32, kind="ExternalInput")
    bo_d = nc.dram_tensor("bo", [D], F32, kind="ExternalInput")
    mask_d = nc.dram_tensor("mask", [P, P], BF16, kind="ExternalInput")
    ident_d = nc.dram_tensor("ident", [P, P], BF16, kind="ExternalInput")
    identr_d = nc.dram_tensor("identr", [P, P], F32R, kind="ExternalInput")
    ones_d = nc.dram_tensor("ones", [P, NTOK // P], BF16,
                            kind="ExternalInput")
    out_d = nc.dram_tensor("out", [TPC, D], F32, kind="ExternalOutput")

    with tile.TileContext(nc) as tc:
        with (
            tc.tile_pool(name="consts", bufs=1) as consts,
            tc.tile_pool(name="acts", bufs=1) as acts,
            tc.tile_pool(name="xin", bufs=3) as xin,
            tc.tile_pool(name="attn", bufs=3) as attn_pool,
            tc.tile_pool(name="small", bufs=2) as small,
            tc.tile_pool(name="outg", bufs=1) as outg,
            tc.tile_pool(name="outp", bufs=2) as outp,
            tc.tile_pool(name="psA", bufs=2, space="PSUM") as psA,
            tc.tile_pool(name="psC", bufs=2, space="PSUM") as psC,
            tc.tile_pool(name="dram", bufs=2, space="DRAM") as dram,
        ):
            # ---- startup: few large DMAs, three issue queues ------------
            wq_sb = consts.tile([P, KO, DSL], BF16, tag="wq")
            wk_sb = consts.tile([P, KO, DSL], BF16, tag="wk")
            wv_sb = consts.tile([P, KO, DSL], BF16, tag="wv")
            bq_sb = consts.tile([P, 1], F32, tag="bq")
            bk_sb = consts.tile([P, 1], F32, tag="bk")
            bv_sb = consts.tile([P, 1], F32, tag="bv")

            # pair-0 x slab in two chunks so the first matmul starts early
            xt0 = xin.tile([P, KO, 2 * QBLK], BF16, tag="xt", name="xt0")
            nc.sync.dma_start(xt0[:, 0:2], xT_d.ap()[:, 0, 0:2])
            nc.scalar.dma_start(
                wq_sb[:], wqT_d.ap().rearrange("(ko p) m -> p ko m", p=P))
            nc.sync.dma_start(xt0[:, 2:KO], xT_d.ap()[:, 0, 2:KO])
            nc.scalar.dma_start(
                wk_sb[:], wkT_d.ap().rearrange("(ko p) m -> p ko m", p=P))
            nc.scalar.dma_start(
                wv_sb[:], wvT_d.ap().rearrange("(ko p) m -> p ko m", p=P))
            nc.gpsimd.dma_start(bq_sb[:], bq_d.ap())
            nc.gpsimd.dma_start(bk_sb[:], bk_d.ap())
            nc.gpsimd.dma_start(bv_sb[:], bv_d.ap())

            identr_sb = consts.tile([P, P], F32R, tag="identr")
            nc.gpsimd.dma_start(identr_sb[:], identr_d.ap())
            ident_sb = consts.tile([P, P], BF16, tag="ident")
            nc.gpsimd.dma_start(ident_sb[:], ident_d.ap())
            mask_sb = consts.tile([P, P], BF16, tag="mask")
            nc.gpsimd.dma_start(mask_sb[:], mask_d.ap())

            qT = acts.tile([P, NTOK], BF16, tag="qT")
            kT = acts.tile([P, NTOK], BF16, tag="kT")
            vT = acts.tile([P, NTOK], F32R, tag="vT")
            v_nat = acts.tile([P, NTOK // P, 2 * DA], BF16, tag="v_nat")
            nc.gpsimd.dma_start(v_nat[:, :, DK], ones_d.ap())
            nc.gpsimd.dma_start(v_nat[:, :, DA + DK], ones_d.ap())

            def proj_pair(i, xt):
                """QKV projections for token slabs 2i, 2i+1.  Bias epilogue
                on VectorE (frees ScalarE for attention exp)."""
                for w_sb, b_sb, dst in ((wq_sb, bq_sb, qT),
                                        (wk_sb, bk_sb, kT),
                                        (wv_sb, bv_sb, vT)):
                    ps = psA.tile([P, 2 * QBLK], F32, tag="sp", name="ps")
                    for ko in range(KO):
                        nc.tensor.matmul(ps[:, 0:QBLK], w_sb[:, ko],
                                         xt[:, ko, 0:QBLK], start=(ko == 0),
                                         stop=(ko == KO - 1))
                        nc.tensor.matmul(ps[:, QBLK:], w_sb[:, ko],
                                         xt[:, ko, QBLK:], start=(ko == 0),
                                         stop=(ko == KO - 1))
                    nc.vector.tensor_scalar_add(dst[:, ts(i, 2 * QBLK)],
                                                ps[:], b_sb[:, 0:1])

            def make_pair(i):
                xt = xin.tile([P, KO, 2 * QBLK], BF16, tag="xt",
                              name=f"xt{i}")
                nc.sync.dma_start(xt[:], xT_d.ap()[:, i])
                proj_pair(i, xt)

            def v_nat_block(j):
                """Transpose one [128,128] vT tile into v_nat (both heads),
                leaving the ones columns intact."""
                ptf = psA.tile([P, 2 * QBLK], F32R, tag="sp", name="ptf")
                pt = ptf[:, :P]
                nc.tensor.transpose(pt[:], vT[:, ts(j, P)], identr_sb[:])
                nc.vector.tensor_copy(v_nat[:, j, 0:DK], pt[:, 0:DK])
                nc.vector.tensor_copy(v_nat[:, j, DA:DA + DK], pt[:, DK:P])

            a2a_in = [dram.tile([NCORES, P, TPG], BF16, tag=f"a2a_in{b}{g}",
                                name=f"a2a_in{b}{g}")
                      for b in range(B) for g in range(G)]
            a2a_out = [dram.tile([NCORES, P, TPG], BF16, tag=f"a2a_out{b}{g}",
                                 name=f"a2a_out{b}{g}")
                       for b in range(B) for g in range(G)]

            def attention_qblock(b, qi):
                q_sl = ts(b * T // QBLK + qi, QBLK)
                nkb = (qi + 1) * (QBLK // KBLK)
                C0 = psC.tile([P, QBLK], F32, tag="ctx0", name="C0")
                C1 = psC.tile([P, QBLK], F32, tag="ctx1", name="C1")

                def emit_ctx(pend):
                    ap_, jjp, doff, st, sp = pend
                    d = max(doff, 0)
                    nc.tensor.matmul(C0[0:DA, d:], v_nat[:, jjp, 0:DA],
                                     ap_[:, d:QBLK], start=st, stop=sp,
                                     skip_group_check=True)
                    nc.tensor.matmul(C1[0:DA, d:], v_nat[:, jjp, DA:2 * DA],
                                     ap_[:, QBLK + d:], start=st, stop=sp,
                                     skip_group_check=True)

                pend = None
                for ki in range(nkb):
                    k_sl = ts(b * T // KBLK + ki, KBLK)
                    jj = b * NKB + ki
                    doff = ki * KBLK - qi * QBLK
                    diag = doff >= 0
                    d = max(doff, 0)
                    sp_t = psA.tile([P, 2 * QBLK], F32, tag="sp", name="sp_t")
                    nc.tensor.matmul(sp_t[:, d:QBLK],
                                     kT[0:DK, k_sl], qT[0:DK, q_sl][:, d:],
                                     start=True, stop=not diag,
                                     tile_position=(0, 0),
                                     skip_group_check=diag)
                    nc.tensor.matmul(sp_t[:, QBLK + d:],
                                     kT[DK:P, k_sl], qT[DK:P, q_sl][:, d:],
                                     start=True, stop=not diag,
                                     tile_position=(64, 0),
                                     skip_group_check=diag)
                    if diag:
                        # triangular -300 bias over the 128-wide diagonal
                        nc.tensor.matmul(sp_t[:, d:d + KBLK], ident_sb[:],
                                         mask_sb[:], start=False, stop=True,
                                         skip_group_check=True)
                        nc.tensor.matmul(sp_t[:, QBLK + d:QBLK + d + KBLK],
                                         ident_sb[:], mask_sb[:],
                                         start=False, stop=True,
                                         skip_group_check=True)
                    a_p = attn_pool.tile([P, 2 * QBLK], BF16, tag="ap",
                                         name="a_p")
                    sp_v = sp_t[:].rearrange("p (h q) -> p h q", h=2)
                    ap_v = a_p[:].rearrange("p (h q) -> p h q", h=2)
                    nc.scalar.activation(ap_v[:, :, d:], sp_v[:, :, d:], EXP)
                    # software pipeline: ctx of the previous k-block issues
                    # after this block's scores, so PE runs ahead of ACT.
                    if pend is not None:
                        emit_ctx(pend)
                    pend = (a_p, jj, doff, ki == 0, ki == nkb - 1)
                emit_ctx(pend)

                # normalize ctx^T by 1/denominator (row 64): pull the
                # denominator rows out of PSUM on ScalarE, broadcast across
                # partitions via a DRAM bounce, then wide VectorE ops.
                den_sb = small.tile([DA, 2 * QBLK], F32, tag="den_sb")
                nc.scalar.copy(den_sb[DK:DA, 0:QBLK], C0[DK:DA])
                nc.scalar.copy(den_sb[DK:DA, QBLK:], C1[DK:DA])
                den_dr = dram.tile([1, 2 * QBLK], F32, tag="den_dr",
                                   name="den_dr")
                nc.scalar.dma_start(den_dr[:], den_sb[DK:DA, :])
                bc_sb = small.tile([DK, 2 * QBLK], F32, tag="bc_sb")
                nc.scalar.dma_start(
                    bc_sb[:], den_dr[:].to_broadcast((DK, 2 * QBLK)))
                rec_sb = small.tile([DK, 2 * QBLK], F32, tag="rec_sb")
                nc.vector.reciprocal_approx_fast(rec_sb[:], bc_sb[:])
                ctx0_sb = small.tile([DK, QBLK], BF16, tag="ctx0_sb")
                ctx1_sb = small.tile([DK, QBLK], BF16, tag="ctx1_sb")
                nc.vector.tensor_mul(ctx0_sb[:], C0[0:DK],
                                     rec_sb[:, 0:QBLK])
                nc.vector.tensor_mul(ctx1_sb[:], C1[0:DK],
                                     rec_sb[:, QBLK:])
                bg = b * G + qi // GQ
                for sub in range(NDST):
                    dst = ((qi % GQ) * QBLK) // TPG + sub
                    nc.sync.dma_start(a2a_in[bg][dst, 0:DK],
                                      ctx0_sb[:, ts(sub, TPG)])
                    nc.sync.dma_start(a2a_in[bg][dst, DK:P],
                                      ctx1_sb[:, ts(sub, TPG)])

            def trigger_coll(bg):
                nc.gpsimd.collective_compute(
                    "AllToAll",
                    mybir.AluOpType.bypass,
                    replica_groups=[list(range(NCORES))],
                    ins=[a2a_in[bg][:].opt()],
                    outs=[a2a_out[bg][:].opt()],
                )

            # wide constants for the tail, loaded mid-kernel so they don't
            # fight the startup DMA burst
            wo_sb = consts.tile([P, KO, D], BF16, tag="wo")
            bo_sb = consts.tile([P, D], F32, tag="bo")

            ctxg = [outg.tile([P, KO, TPG], BF16, tag=f"ctxg{bg}",
                              name=f"ctxg{bg}") for bg in range(B * G)]

            def gather_group(bg):
                nc.sync.dma_start(
                    ctxg[bg][:], a2a_out[bg][:].rearrange("j p t -> p j t"))

            def outproj_tb(bg, tb):
                b, g = bg // G, bg % G
                po = psA.tile([P, 2 * QBLK], F32, tag="sp", name="po")
                for ko in range(KO):
                    nc.tensor.matmul(po[0:TW, 0:QBLK],
                                     ctxg[bg][:, ko, ts(tb, TW)],
                                     wo_sb[:, ko, 0:QBLK],
                                     start=(ko == 0), stop=(ko == KO - 1))
                    nc.tensor.matmul(po[0:TW, QBLK:],
                                     ctxg[bg][:, ko, ts(tb, TW)],
                                     wo_sb[:, ko, QBLK:],
                                     start=(ko == 0), stop=(ko == KO - 1))
                o_sb = outp.tile([TW, D], F32, tag="o_sb", name=f"o{bg}{tb}")
                nc.vector.tensor_add(o_sb[:], po[0:TW, :], bo_sb[:TW])
                row = b * TPB + g * TPG + tb * TW
                nc.gpsimd.dma_start(out_d.ap()[row:row + TW, :], o_sb[:])

            # ---- phase plan ---------------------------------------------
            half_pairs = NPAIR // B  # proj pairs per batch
            proj_pair(0, xt0)
            for i in range(1, half_pairs):
                make_pair(i)
            for j in range(NTOK // P // B):
                v_nat_block(j)

            late = []
            for i in range(half_pairs, NPAIR):
                late.append(lambda i=i: make_pair(i))
            late.append(lambda: nc.scalar.dma_start(
                wo_sb[:], woT_d.ap().rearrange("(ko p) m -> p ko m", p=P)))
            late.append(lambda: nc.scalar.dma_start(
                bo_sb[:], bo_d.ap()[None, :].to_broadcast((P, D))))
            for j0 in range(NTOK // P // B, NTOK // P, 4):
                late.append(lambda j0=j0: [v_nat_block(j)
                                           for j in range(j0, j0 + 4)])

            # batch-0 attention, interleaving batch-1 prep into PE gaps;
            # each token group's AllToAll fires as soon as it completes
            for qi in range(NQB):
                attention_qblock(0, qi)
                if qi % GQ == GQ - 1:
                    trigger_coll(qi // GQ)
                nlate = max(1, (len(late) + NQB - 1 - qi) // (NQB - qi))
                for _ in range(min(nlate, len(late))):
                    late.pop(0)()
            while late:
                late.pop(0)()

            # batch-1 attention; batch-0 exchanges + output projection
            # quarters hide underneath it
            for qi in range(NQB):
                attention_qblock(1, qi)
                if qi % GQ == GQ - 1:
                    trigger_coll(G + qi // GQ)
                if qi < G:
                    gather_group(qi)
                if 1 <= qi <= G:
                    for tb in range(TBG):
                        outproj_tb(qi - 1, tb)
            # flush batch-0 leftovers (small NQB), then the batch-1 tail
            for bg in range(min(G, NQB - 1), G):
                for tb in range(TBG):
                    outproj_tb(bg, tb)
            for g in range(G):
                gather_group(G + g)
                for tb in range(TBG):
                    outproj_tb(G + g, tb)

    nc.compile()
    return nc


_NC_CACHE = {}


def _get_nc(B, T):
    key = (B, T)
    if key not in _NC_CACHE:
        _NC_CACHE[key] = build_nc(B, T)
    return _NC_CACHE[key]


def make_in_maps(x, Wq, bq, Wk, bk, Wv, bv, Wo, bo):
    B, T, _ = x.shape
    NTOK = B * T
    NPAIR = NTOK // (2 * QBLK)
    KO = D // P
    x = np.asarray(x, np.float32)
    # [D, NTOK] -> [p, pair, ko, t]: one contiguous DMA descriptor per
    # partition per (pair, ko) chunk.
    xT = x.reshape(NTOK, D).T  # [D, NTOK]
    xT_t = np.ascontiguousarray(
        xT.reshape(KO, P, NPAIR, 2 * QBLK).transpose(1, 2, 0, 3)
    ).astype(BF16_NP)
    woT = np.ascontiguousarray(np.asarray(Wo, np.float32).T).astype(BF16_NP)
    bo = np.asarray(bo, np.float32)
    # triangular -300 bias for the 128-wide diagonal block: keep (0) where
    # the local query column c is >= the local key partition p.
    keep = np.arange(P)[None, :] >= np.arange(P)[:, None]
    mask = np.where(keep, 0.0, -300.0).astype(BF16_NP)
    ident = np.eye(P, dtype=np.float32)
    ones = np.ones((P, NTOK // P), BF16_NP)
    in_maps = []
    for c in range(NCORES):
        sl = slice(DSL * c, DSL * (c + 1))
        in_maps.append({
            "xT": xT_t,
            "wqT": np.ascontiguousarray(
                np.asarray(Wq, np.float32)[sl].T * 0.125).astype(BF16_NP),
            "wkT": np.ascontiguousarray(
                np.asarray(Wk, np.float32)[sl].T).astype(BF16_NP),
            "wvT": np.ascontiguousarray(
                np.asarray(Wv, np.float32)[sl].T).astype(BF16_NP),
            "woT": woT,
            "bq": (np.asarray(bq, np.float32)[sl] * 0.125).reshape(DSL, 1),
            "bk": np.asarray(bk, np.float32)[sl].reshape(DSL, 1),
            "bv": np.asarray(bv, np.float32)[sl].reshape(DSL, 1),
            "bo": bo,
            "mask": mask,
            "ident": ident.astype(BF16_NP),
            "identr": ident,
            "ones": ones,
        })
    return in_maps


LAST_RESULTS = None


def assemble_out(per_core, B, T):
    """per_core[c] is [TPC, D] with rows ordered (batch, group, token)."""
    TPB = T // NCORES
    TPG = TPB // G
    out = np.empty((B, T, D), np.float32)
    for c in range(NCORES):
        oc = per_core[c]
        for b in range(B):
            for g in range(G):
                r = b * TPB + g * TPG
                t = g * (T // G) + c * TPG
                out[b, t:t + TPG] = oc[r:r + TPG]
    return out


def kernel(x, Wq, bq, Wk, bk, Wv, bv, Wo, bo, trace=False, trace_cores=None):
    global LAST_RESULTS
    B, T, _ = x.shape
    assert B == 2
    nc = _get_nc(B, T)
    in_maps = make_in_maps(x, Wq, bq, Wk, bk, Wv, bv, Wo, bo)
    kw = {}
    if trace:
        kw = dict(trace=True, trace_cores=trace_cores)
    res = bass_utils.run_bass_kernel_spmd(nc, in_maps,
                                          core_ids=list(range(NCORES)), **kw)
    LAST_RESULTS = res
    return assemble_out([res.results[c]["out"] for c in range(NCORES)], B, T)


# revision 10
# speedup vs baseline: 1.5050x; 1.0410x over previous
"""Multi-head attention (B=2, T=2048, D=1024, H=16, causal) on 8 TRN2 NeuronCores.

Sharding (tensor-parallel heads + token-parallel epilogue):
  - Core c owns heads (2c, 2c+1) -> a 128-wide slice of the QKV output dim.
  - bf16 operands everywhere on the matmul path (f32 PSUM accumulation);
    x / weights are cast to bf16 on the host, halving HBM traffic.
  - Startup DMAs are issue-rate bound (~650ns per dma_start on a queue), so
    the critical path uses few, large dma_starts spread over three issue
    queues (sync: x slabs, scalar: weights, gpsimd: small constants).
  - QKV projections: qT/kT [128, B*T] bf16 feature-major; bias epilogues on
    VectorE (tensor_scalar_add) so ScalarE is reserved for attention exp.
  - Attention: streaming over 128-wide key blocks, transposed score tiles
    S^T [k, q] for both heads in one [128, 1024] PSUM pair; on diagonal
    blocks only the valid suffix [doff, 512) is computed and a 128-wide
    triangular -300 bias tile is accumulated on top (exp underflows to 0);
    one (narrowed) exp per k-block on ScalarE; ctx^T accumulates with an
    appended ones-column in v so row 64 of the accumulator is the softmax
    denominator.  The k-loop is software-pipelined (ctx of block k issues
    after scores of block k+1).
  - Softmax normalize: denominator rows leave PSUM via ScalarE copies, are
    DMA-broadcast across partitions through a DRAM bounce, inverted with
    reciprocal_approx_fast, and applied as wide [64, N] VectorE multiplies.
  - The ctx AllToAll is split into FOUR collectives (2 batches x 2 token
    groups), each triggered as soon as its qblocks finish, so exchanges and
    the token-sharded output projection overlap attention compute; only the
    last quarter remains in the tail.  Host reassembles the token layout.
"""

import numpy as np
import ml_dtypes

import concourse.bacc as bacc
import concourse.bass as bass
import concourse.mybir as mybir
import concourse.tile as tile
from concourse import bass_utils
from concourse.bass import ts

D = 1024
H = 16
DK = D // H  # 64
NCORES = 8
HPC = H // NCORES  # heads per core = 2
DSL = HPC * DK  # per-core QKV output slice = 128
P = 128
QBLK = 512
KBLK = 128
DA = DK + 1  # 65: head dim + ones column (softmax denominator row)
G = 2  # token groups per batch (collective split granularity)

F32 = mybir.dt.float32
F32R = mybir.dt.float32r
BF16 = mybir.dt.bfloat16
EXP = mybir.ActivationFunctionType.Exp

BF16_NP = ml_dtypes.bfloat16


def build_nc(B=2, T=2048):
    """Build the SPMD Bass module (identical program on all 8 cores)."""
    NTOK = B * T
    TPC = NTOK // NCORES  # tokens per core in the output projection
    TPB = T // NCORES  # tokens per core per batch
    TPG = TPB // G  # tokens per core per (batch, group)
    KO = D // P  # 8 contraction chunks
    NKB = T // KBLK  # key blocks per batch
    NQB = T // QBLK  # query blocks per batch
    GQ = NQB // G  # q-blocks per group
    NSLAB = NTOK // QBLK  # x token slabs
    NPAIR = NSLAB // 2
    NDST = QBLK // TPG  # destination cores covered by one q-block
    TW = min(P, TPG)  # outproj token-tile width
    TBG = TPG // TW  # outproj token tiles per (batch, group)

    nc = bacc.Bacc("TRN2", target_bir_lowering=False, debug=False,
                   num_devices=NCORES)

    # ---- DRAM I/O ------------------------------------------------------
    xT_d = nc.dram_tensor("xT", [P, NPAIR, KO, 2 * QBLK], BF16,
                          kind="ExternalInput")
    wqT_d = nc.dram_tensor("wqT", [P, KO, DSL], BF16, kind="ExternalInput")
    wkT_d = nc.dram_tensor("wkT", [P, KO, DSL], BF16, kind="ExternalInput")
    wvT_d = nc.dram_tensor("wvT", [P, KO, DSL], BF16, kind="ExternalInput")
    woT_d = nc.dram_tensor("woT", [P, KO, D], BF16, kind="ExternalInput")
    bq_d = nc.dram_tensor("bq", [DSL, 1], F32, kind="ExternalInput")
    bk_d = nc.dram_tensor("bk", [DSL, 1], F32, kind="ExternalInput")
    bv_d = nc.dram_tensor("bv", [DSL, 1], F32, kind="ExternalInput")
    bo_d = nc.dram_tensor("bo", [D], F32, kind="ExternalInput")
    mask_d = nc.dram_tensor("mask", [P, P], BF16, kind="ExternalInput")
    ident_d = nc.dram_tensor("ident", [P, P], BF16, kind="ExternalInput")
    identr_d = nc.dram_tensor("identr", [P, P], F32R, kind="ExternalInput")
    out_d = nc.dram_tensor("out", [TPC, D], F32, kind="ExternalOutput")

    with tile.TileContext(nc) as tc:
        with (
            tc.tile_pool(name="consts", bufs=1) as consts,
            tc.tile_pool(name="acts", bufs=1) as acts,
            tc.tile_pool(name="xin", bufs=3) as xin,
            tc.tile_pool(name="attn", bufs=3) as attn_pool,
            tc.tile_pool(name="small", bufs=2) as small,
            tc.tile_pool(name="outg", bufs=1) as outg,
            tc.tile_pool(name="outp", bufs=2) as outp,
            tc.tile_pool(name="psA", bufs=2, space="PSUM") as psA,
            tc.tile_pool(name="psC", bufs=2, space="PSUM") as psC,
            tc.tile_pool(name="dram", bufs=2, space="DRAM") as dram,
        ):
            # ---- startup: few large DMAs, three issue queues ------------
            wq_sb = consts.tile([P, KO, DSL], BF16, tag="wq")
            wk_sb = consts.tile([P, KO, DSL], BF16, tag="wk")
            wv_sb = consts.tile([P, KO, DSL], BF16, tag="wv")
            bq_sb = consts.tile([P, 1], F32, tag="bq")
            bk_sb = consts.tile([P, 1], F32, tag="bk")
            bv_sb = consts.tile([P, 1], F32, tag="bv")

            # pair-0 x slab in two chunks so the first matmul starts early
            xt0 = xin.tile([P, KO, 2 * QBLK], BF16, tag="xt", name="xt0")
            nc.sync.dma_start(xt0[:, 0:2], xT_d.ap()[:, 0, 0:2])
            nc.scalar.dma_start(wq_sb[:], wqT_d.ap())
            nc.sync.dma_start(xt0[:, 2:KO], xT_d.ap()[:, 0, 2:KO])
            nc.scalar.dma_start(wk_sb[:], wkT_d.ap())
            nc.scalar.dma_start(wv_sb[:], wvT_d.ap())
            nc.gpsimd.dma_start(bq_sb[:], bq_d.ap())
            nc.gpsimd.dma_start(bk_sb[:], bk_d.ap())
            nc.gpsimd.dma_start(bv_sb[:], bv_d.ap())

            identr_sb = consts.tile([P, P], F32R, tag="identr")
            nc.gpsimd.dma_start(identr_sb[:], identr_d.ap())
            ident_sb = consts.tile([P, P], BF16, tag="ident")
            nc.gpsimd.dma_start(ident_sb[:], ident_d.ap())
            mask_sb = consts.tile([P, P], BF16, tag="mask")
            nc.gpsimd.dma_start(mask_sb[:], mask_d.ap())

            qT = acts.tile([P, NTOK], BF16, tag="qT")
            kT = acts.tile([P, NTOK], BF16, tag="kT")
            vT = acts.tile([P, NTOK], F32R, tag="vT")
            v_nat = acts.tile([P, NTOK // P, 2 * DA], BF16, tag="v_nat")
            nc.gpsimd.memset(v_nat[:, :, DK], 1.0)
            nc.gpsimd.memset(v_nat[:, :, DA + DK], 1.0)

            # tiny dummy collective to absorb the first-collective arming
            # cost (~11us) during the startup phase
            warm_in = dram.tile([NCORES, 2], BF16, tag="warm_in",
                                name="warm_in")
            warm_out = dram.tile([NCORES, 2], BF16, tag="warm_out",
                                 name="warm_out")
            nc.gpsimd.dma_start(warm_in[:], mask_d.ap()[0:NCORES, 0:2])
            nc.gpsimd.collective_compute(
                "AllToAll",
                mybir.AluOpType.bypass,
                replica_groups=[list(range(NCORES))],
                ins=[warm_in[:].opt()],
                outs=[warm_out[:].opt()],
            )

            def proj_pair(i, xt):
                """QKV projections for token slabs 2i, 2i+1.  Bias epilogue
                on VectorE (frees ScalarE for attention exp)."""
                for w_sb, b_sb, dst in ((wq_sb, bq_sb, qT),
                                        (wk_sb, bk_sb, kT),
                                        (wv_sb, bv_sb, vT)):
                    ps = psA.tile([P, 2 * QBLK], F32, tag="sp", name="ps")
                    for ko in range(KO):
                        nc.tensor.matmul(ps[:, 0:QBLK], w_sb[:, ko],
                                         xt[:, ko, 0:QBLK], start=(ko == 0),
                                         stop=(ko == KO - 1))
                        nc.tensor.matmul(ps[:, QBLK:], w_sb[:, ko],
                                         xt[:, ko, QBLK:], start=(ko == 0),
                                         stop=(ko == KO - 1))
                    nc.vector.tensor_scalar_add(dst[:, ts(i, 2 * QBLK)],
                                                ps[:], b_sb[:, 0:1])

            def make_pair(i):
                xt = xin.tile([P, KO, 2 * QBLK], BF16, tag="xt",
                              name=f"xt{i}")
                nc.sync.dma_start(xt[:], xT_d.ap()[:, i])
                proj_pair(i, xt)

            def v_nat_block(j):
                """Transpose one [128,128] vT tile into v_nat (both heads),
                leaving the ones columns intact."""
                ptf = psA.tile([P, 2 * QBLK], F32R, tag="sp", name="ptf")
                pt = ptf[:, :P]
                nc.tensor.transpose(pt[:], vT[:, ts(j, P)], identr_sb[:])
                nc.vector.tensor_copy(v_nat[:, j, 0:DK], pt[:, 0:DK])
                nc.vector.tensor_copy(v_nat[:, j, DA:DA + DK], pt[:, DK:P])

            a2a_in = [dram.tile([NCORES, P, TPG], BF16, tag=f"a2a_in{b}{g}",
                                name=f"a2a_in{b}{g}")
                      for b in range(B) for g in range(G)]
            a2a_out = [dram.tile([NCORES, P, TPG], BF16, tag=f"a2a_out{b}{g}",
                                 name=f"a2a_out{b}{g}")
                       for b in range(B) for g in range(G)]

            def attention_qblock(b, qi):
                q_sl = ts(b * T // QBLK + qi, QBLK)
                nkb = (qi + 1) * (QBLK // KBLK)
                C0 = psC.tile([P, QBLK], F32, tag="ctx0", name="C0")
                C1 = psC.tile([P, QBLK], F32, tag="ctx1", name="C1")

                def emit_ctx(pend):
                    ap_, jjp, doff, st, sp = pend
                    d = max(doff, 0)
                    nc.tensor.matmul(C0[0:DA, d:], v_nat[:, jjp, 0:DA],
                                     ap_[:, d:QBLK], start=st, stop=sp,
                                     skip_group_check=True)
                    nc.tensor.matmul(C1[0:DA, d:], v_nat[:, jjp, DA:2 * DA],
                                     ap_[:, QBLK + d:], start=st, stop=sp,
                                     skip_group_check=True)

                pend = None
                for ki in range(nkb):
                    k_sl = ts(b * T // KBLK + ki, KBLK)
                    jj = b * NKB + ki
                    doff = ki * KBLK - qi * QBLK
                    diag = doff >= 0
                    d = max(doff, 0)
                    sp_t = psA.tile([P, 2 * QBLK], F32, tag="sp", name="sp_t")
                    nc.tensor.matmul(sp_t[:, d:QBLK],
                                     kT[0:DK, k_sl], qT[0:DK, q_sl][:, d:],
                                     start=True, stop=not diag,
                                     tile_position=(0, 0),
                                     skip_group_check=diag)
                    nc.tensor.matmul(sp_t[:, QBLK + d:],
                                     kT[DK:P, k_sl], qT[DK:P, q_sl][:, d:],
                                     start=True, stop=not diag,
                                     tile_position=(64, 0),
                                     skip_group_check=diag)
                    if diag:
                        # triangular -300 bias over the 128-wide diagonal
                        nc.tensor.matmul(sp_t[:, d:d + KBLK], ident_sb[:],
                                         mask_sb[:], start=False, stop=True,
                                         skip_group_check=True)
                        nc.tensor.matmul(sp_t[:, QBLK + d:QBLK + d + KBLK],
                                         ident_sb[:], mask_sb[:],
                                         start=False, stop=True,
                                         skip_group_check=True)
                    a_p = attn_pool.tile([P, 2 * QBLK], BF16, tag="ap",
                                         name="a_p")
                    sp_v = sp_t[:].rearrange("p (h q) -> p h q", h=2)
                    ap_v = a_p[:].rearrange("p (h q) -> p h q", h=2)
                    nc.scalar.activation(ap_v[:, :, d:], sp_v[:, :, d:], EXP)
                    # software pipeline: ctx of the previous k-block issues
                    # after this block's scores, so PE runs ahead of ACT.
                    if pend is not None:
                        emit_ctx(pend)
                    pend = (a_p, jj, doff, ki == 0, ki == nkb - 1)
                emit_ctx(pend)

                # normalize ctx^T by 1/denominator (row 64): pull the
                # denominator rows out of PSUM on ScalarE, broadcast across
                # partitions via a DRAM bounce, then wide VectorE ops.
                den_sb = small.tile([DA, 2 * QBLK], F32, tag="den_sb")
                nc.scalar.copy(den_sb[DK:DA, 0:QBLK], C0[DK:DA])
                nc.scalar.copy(den_sb[DK:DA, QBLK:], C1[DK:DA])
                den_dr = dram.tile([1, 2 * QBLK], F32, tag="den_dr",
                                   name="den_dr")
                nc.gpsimd.dma_start(den_dr[:], den_sb[DK:DA, :])
                bc_sb = small.tile([DK, 2 * QBLK], F32, tag="bc_sb")
                nc.gpsimd.dma_start(
                    bc_sb[:], den_dr[:].to_broadcast((DK, 2 * QBLK)))
                rec_sb = small.tile([DK, 2 * QBLK], F32, tag="rec_sb")
                nc.vector.reciprocal_approx_fast(rec_sb[:], bc_sb[:])
                ctx0_sb = small.tile([DK, QBLK], BF16, tag="ctx0_sb")
                ctx1_sb = small.tile([DK, QBLK], BF16, tag="ctx1_sb")
                nc.vector.tensor_mul(ctx0_sb[:], C0[0:DK],
                                     rec_sb[:, 0:QBLK])
                nc.vector.tensor_mul(ctx1_sb[:], C1[0:DK],
                                     rec_sb[:, QBLK:])
                bg = b * G + qi // GQ
                for sub in range(NDST):
                    dst = ((qi % GQ) * QBLK) // TPG + sub
                    nc.sync.dma_start(a2a_in[bg][dst, 0:DK],
                                      ctx0_sb[:, ts(sub, TPG)])
                    nc.sync.dma_start(a2a_in[bg][dst, DK:P],
                                      ctx1_sb[:, ts(sub, TPG)])

            def trigger_coll(bg):
                nc.gpsimd.collective_compute(
                    "AllToAll",
                    mybir.AluOpType.bypass,
                    replica_groups=[list(range(NCORES))],
                    ins=[a2a_in[bg][:].opt()],
                    outs=[a2a_out[bg][:].opt()],
                )

            # wide constants for the tail, loaded mid-kernel so they don't
            # fight the startup DMA burst
            wo_sb = consts.tile([P, KO, D], BF16, tag="wo")
            bo_sb = consts.tile([P, D], F32, tag="bo")

            ctxg = [outg.tile([P, KO, TPG], BF16, tag=f"ctxg{bg}",
                              name=f"ctxg{bg}") for bg in range(B * G)]

            def gather_group(bg):
                nc.sync.dma_start(
                    ctxg[bg][:], a2a_out[bg][:].rearrange("j p t -> p j t"))

            def outproj_tb(bg, tb):
                b, g = bg // G, bg % G
                po = psA.tile([P, 2 * QBLK], F32, tag="sp", name="po")
                for ko in range(KO):
                    nc.tensor.matmul(po[0:TW, 0:QBLK],
                                     ctxg[bg][:, ko, ts(tb, TW)],
                                     wo_sb[:, ko, 0:QBLK],
                                     start=(ko == 0), stop=(ko == KO - 1))
                    nc.tensor.matmul(po[0:TW, QBLK:],
                                     ctxg[bg][:, ko, ts(tb, TW)],
                                     wo_sb[:, ko, QBLK:],
                                     start=(ko == 0), stop=(ko == KO - 1))
                o_sb = outp.tile([TW, D], F32, tag="o_sb", name=f"o{bg}{tb}")
                nc.vector.tensor_add(o_sb[:], po[0:TW, :], bo_sb[:TW])
                row = b * TPB + g * TPG + tb * TW
                nc.gpsimd.dma_start(out_d.ap()[row:row + TW, :], o_sb[:])

            # ---- phase plan ---------------------------------------------
            half_pairs = NPAIR // B  # proj pairs per batch
            proj_pair(0, xt0)
            for i in range(1, half_pairs):
                make_pair(i)
            for j in range(NTOK // P // B):
                v_nat_block(j)

            late = []
            for i in range(half_pairs, NPAIR):
                late.append(lambda i=i: make_pair(i))
            late.append(lambda: nc.scalar.dma_start(
                wo_sb[:], woT_d.ap()))
            late.append(lambda: nc.scalar.dma_start(
                bo_sb[:], bo_d.ap()[None, :].to_broadcast((P, D))))
            for j0 in range(NTOK // P // B, NTOK // P, 4):
                late.append(lambda j0=j0: [v_nat_block(j)
                                           for j in range(j0, j0 + 4)])

            # batch-0 attention, interleaving batch-1 prep into PE gaps;
            # each token group's AllToAll fires as soon as it completes
            for qi in range(NQB):
                attention_qblock(0, qi)
                if qi % GQ == GQ - 1:
                    trigger_coll(qi // GQ)
                nlate = max(1, (len(late) + NQB - 1 - qi) // (NQB - qi))
                for _ in range(min(nlate, len(late))):
                    late.pop(0)()
            while late:
                late.pop(0)()

            # batch-1 attention; batch-0 exchanges + output projection
            # quarters hide underneath it
            for qi in range(NQB):
                attention_qblock(1, qi)
                if qi % GQ == GQ - 1:
                    trigger_coll(G + qi // GQ)
                if qi < G:
                    gather_group(qi)
                if 1 <= qi <= G:
                    for tb in range(TBG):
                        outproj_tb(qi - 1, tb)
            # flush batch-0 leftovers (small NQB), then the batch-1 tail
            for bg in range(min(G, NQB - 1), G):
                for tb in range(TBG):
                    outproj_tb(bg, tb)
            for g in range(G):
                gather_group(G + g)
                for tb in range(TBG):
                    outproj_tb(G + g, tb)

    nc.compile()
    return nc


_NC_CACHE = {}


def _get_nc(B, T):
    key = (B, T)
    if key not in _NC_CACHE:
        _NC_CACHE[key] = build_nc(B, T)
    return _NC_CACHE[key]


def make_in_maps(x, Wq, bq, Wk, bk, Wv, bv, Wo, bo):
    B, T, _ = x.shape
    NTOK = B * T
    NPAIR = NTOK // (2 * QBLK)
    KO = D // P
    x = np.asarray(x, np.float32)
    # [D, NTOK] -> [p, pair, ko, t]: one contiguous DMA descriptor per
    # partition per (pair, ko) chunk.
    xT = x.reshape(NTOK, D).T  # [D, NTOK]
    xT_t = np.ascontiguousarray(
        xT.reshape(KO, P, NPAIR, 2 * QBLK).transpose(1, 2, 0, 3)
    ).astype(BF16_NP)
    woT = np.ascontiguousarray(
        np.asarray(Wo, np.float32).T.reshape(KO, P, D).transpose(1, 0, 2)
    ).astype(BF16_NP)
    bo = np.asarray(bo, np.float32)
    # triangular -300 bias for the 128-wide diagonal block: keep (0) where
    # the local query column c is >= the local key partition p.
    keep = np.arange(P)[None, :] >= np.arange(P)[:, None]
    mask = np.where(keep, 0.0, -300.0).astype(BF16_NP)
    ident = np.eye(P, dtype=np.float32)
    in_maps = []
    for c in range(NCORES):
        sl = slice(DSL * c, DSL * (c + 1))
        in_maps.append({
            "xT": xT_t,
            "wqT": np.ascontiguousarray(
                (np.asarray(Wq, np.float32)[sl].T * 0.125)
                .reshape(KO, P, DSL).transpose(1, 0, 2)).astype(BF16_NP),
            "wkT": np.ascontiguousarray(
                np.asarray(Wk, np.float32)[sl].T
                .reshape(KO, P, DSL).transpose(1, 0, 2)).astype(BF16_NP),
            "wvT": np.ascontiguousarray(
                np.asarray(Wv, np.float32)[sl].T
                .reshape(KO, P, DSL).transpose(1, 0, 2)).astype(BF16_NP),
            "woT": woT,
            "bq": (np.asarray(bq, np.float32)[sl] * 0.125).reshape(DSL, 1),
            "bk": np.asarray(bk, np.float32)[sl].reshape(DSL, 1),
            "bv": np.asarray(bv, np.float32)[sl].reshape(DSL, 1),
            "bo": bo,
            "mask": mask,
            "ident": ident.astype(BF16_NP),
            "identr": ident,
        })
    return in_maps


LAST_RESULTS = None


def assemble_out(per_core, B, T):
    """per_core[c] is [TPC, D] with rows ordered (batch, group, token)."""
    TPB = T // NCORES
    TPG = TPB // G
    out = np.empty((B, T, D), np.float32)
    for c in range(NCORES):
        oc = per_core[c]
        for b in range(B):
            for g in range(G):
                r = b * TPB + g * TPG
                t = g * (T // G) + c * TPG
                out[b, t:t + TPG] = oc[r:r + TPG]
    return out


def kernel(x, Wq, bq, Wk, bk, Wv, bv, Wo, bo, trace=False, trace_cores=None):
    global LAST_RESULTS
    B, T, _ = x.shape
    assert B == 2
    nc = _get_nc(B, T)
    in_maps = make_in_maps(x, Wq, bq, Wk, bk, Wv, bv, Wo, bo)
    kw = {}
    if trace:
        kw = dict(trace=True, trace_cores=trace_cores)
    res = bass_utils.run_bass_kernel_spmd(nc, in_maps,
                                          core_ids=list(range(NCORES)), **kw)
    LAST_RESULTS = res
    return assemble_out([res.results[c]["out"] for c in range(NCORES)], B, T)
